# revision 10
# baseline (speedup 1.0000x reference)
"""GNN mean-aggregation + 2-layer MLP on 8 Trainium2 NeuronCores.

Reference computation (see problem):
    rows = [i;j], cols = [j;i]                      (symmetrized COO)
    agg[n]  = mean over entries (n, c) of conical[c]   (deg clamped to 1)
    out     = relu([radial | agg] @ W1 + b1) @ W2 + b2

Strategy (nodes sharded 8 ways, MLP weights replicated):
  Host: symmetrize edges, degree-sort nodes (descending), build a padded
  CSR whose pad width K_g is uniform within each "group" of 8 tiles x 128
  nodes (one tile per core), so all 8 cores run the SAME program with
  identical shapes.  Per-core aux tensors: neighbor-index table
  idx[128, S], feature-major radial block, 1/deg, split W1.
  Device (per core): per 512-node block, one wide indirect-DMA gather
  [128, sum(K)*16] from the conical table (the memory-bound part), DVE
  tree-reduce to per-node sums, scale by 1/deg, PE-transpose to
  feature-major, MLP with stationary weights (bias via rank-1 matmul),
  relu on ACT, node-major DMA out.  Host inverse-permutes shards.
"""

import math

import numpy as np

N_CORES = 8
P = 128
GROUP = N_CORES * P  # 1024 nodes per group (one 128-tile per core)
FH = 16  # radial / conical half-width
F = 32
HID = 128
BLOCK_GROUPS = 4  # groups fused per gather instruction / MLP block


# ---------------------------------------------------------------- host prep


def _host_prep(x, edge_index):
    N = x.shape[0]
    i = edge_index[0].astype(np.int64)
    j = edge_index[1].astype(np.int64)
    rows = np.concatenate([i, j])
    cols = np.concatenate([j, i]).astype(np.int32)
    deg = np.bincount(rows, minlength=N)

    order = np.argsort(-deg, kind="stable")  # new id -> orig id
    deg_sorted = deg[order]

    eorder = np.argsort(rows, kind="stable")
    sorted_cols = cols[eorder]
    row_ptr = np.zeros(N + 1, np.int64)
    row_ptr[1:] = np.cumsum(deg)

    n_groups = math.ceil(N / GROUP)
    Ks = [max(int(deg_sorted[g * GROUP]), 1) for g in range(n_groups)]
    S = sum(Ks)
    ncols = n_groups * P  # rows per core shard

    idx_all = np.full((N_CORES, P, S), N, np.int32)  # N = zero row
    invdeg_all = np.ones((N_CORES, P, n_groups), np.float32)
    radial_all = np.zeros((N_CORES, FH, ncols), np.float32)

    total = sorted_cols.shape[0]
    off = 0
    for g in range(n_groups):
        K = Ks[g]
        lo = g * GROUP
        hi = min(lo + GROUP, N)
        n_real = hi - lo
        orig = order[lo:hi]
        d = deg_sorted[lo:hi]
        pos = row_ptr[orig][:, None] + np.arange(K)[None, :]
        mask = np.arange(K)[None, :] < d[:, None]
        vals = np.where(mask, sorted_cols[np.minimum(pos, total - 1)], N)
        blk = np.full((GROUP, K), N, np.int32)
        blk[:n_real] = vals
        idx_all[:, :, off : off + K] = blk.reshape(N_CORES, P, K)

        dd = np.ones((GROUP,), np.float32)
        dd[:n_real] = (1.0 / np.maximum(d, 1)).astype(np.float32)
        invdeg_all[:, :, g] = dd.reshape(N_CORES, P)

        rr = np.zeros((GROUP, FH), np.float32)
        rr[:n_real] = x[orig, :FH]
        radial_all[:, :, g * P : (g + 1) * P] = rr.reshape(N_CORES, P, FH).transpose(
            0, 2, 1
        )
        off += K

    table = np.concatenate([x[:, FH:F], np.zeros((1, FH), np.float32)], axis=0)
    table = np.ascontiguousarray(table, dtype=np.float32)
    return dict(
        order=order,
        Ks=Ks,
        S=S,
        n_groups=n_groups,
        ncols=ncols,
        idx_all=idx_all,
        invdeg_all=invdeg_all,
        radial_all=radial_all,
        table=table,
    )


# ------------------------------------------------------------- bass program


def build_program(Ks, ncols, table_rows):
    import concourse.bass as bass
    import concourse.tile as tile
    from concourse import bacc, mybir

    f32 = mybir.dt.float32
    i32 = mybir.dt.int32
    AF = mybir.ActivationFunctionType
    S = sum(Ks)
    n_groups = len(Ks)

    # Bacc (not raw Bass): its compile() splits multi-wait instructions into
    # event semaphores to satisfy TRN2's 1-wait-per-instruction constraint.
    # 4 SWDGE queues: single-queue descriptor generation caps indirect DMAs
    # at ~1.4us/instruction; spreading them round-robin lifts that.
    nc = bacc.Bacc(None, num_swdge_queues=4, dynamic_dma_scratch_size=65536)
    n_gathers = 0
    table = nc.dram_tensor("table", [table_rows, FH], f32, kind="ExternalInput")
    idxs = nc.dram_tensor("idxs", [P, S], i32, kind="ExternalInput")
    radial = nc.dram_tensor("radial", [FH, ncols], f32, kind="ExternalInput")
    invdeg = nc.dram_tensor("invdeg", [P, n_groups], f32, kind="ExternalInput")
    w1a = nc.dram_tensor("w1a", [FH, HID], f32, kind="ExternalInput")
    w1b = nc.dram_tensor("w1b", [FH, HID], f32, kind="ExternalInput")
    w2 = nc.dram_tensor("w2", [HID, F], f32, kind="ExternalInput")
    b1 = nc.dram_tensor("b1", [HID, 1], f32, kind="ExternalInput")
    b2 = nc.dram_tensor("b2", [1, F], f32, kind="ExternalInput")
    out = nc.dram_tensor("out", [ncols, F], f32, kind="ExternalOutput")

    blocks = []
    g0 = 0
    while g0 < n_groups:
        gs = min(BLOCK_GROUPS, n_groups - g0)
        blocks.append((g0, gs))
        g0 += gs

    with tile.TileContext(nc) as tc:
        with (
            tc.tile_pool(name="res", bufs=1) as res,
            tc.tile_pool(name="work", bufs=3) as work,
            tc.tile_pool(name="mlp", bufs=2) as mlp,
            tc.tile_pool(name="psum", bufs=2, space="PSUM") as psum,
        ):
            idxs_sb = res.tile([P, S], i32)
            nc.sync.dma_start(out=idxs_sb[:], in_=idxs[:])
            radial_sb = res.tile([FH, ncols], f32)
            nc.sync.dma_start(out=radial_sb[:], in_=radial[:])
            invdeg_sb = res.tile([P, n_groups], f32)
            nc.sync.dma_start(out=invdeg_sb[:], in_=invdeg[:])
            w1a_sb = res.tile([FH, HID], f32)
            nc.sync.dma_start(out=w1a_sb[:], in_=w1a[:])
            w1b_sb = res.tile([FH, HID], f32)
            nc.sync.dma_start(out=w1b_sb[:], in_=w1b[:])
            w2_sb = res.tile([HID, F], f32)
            nc.sync.dma_start(out=w2_sb[:], in_=w2[:])
            b1_sb = res.tile([HID, 1], f32)
            nc.sync.dma_start(out=b1_sb[:], in_=b1[:])
            b2_sb = res.tile([1, F], f32)
            nc.sync.dma_start(out=b2_sb[:], in_=b2[:])
            # identity's final writer must be DVE so PE transposes carry one
            # DVE wait instead of an extra Pool wait (HW limit: 2 waits/inst)
            from concourse.masks import make_identity

            ident_tmp = res.tile([P, P], f32)
            make_identity(nc, ident_tmp[:])
            ident_sb = res.tile([P, P], f32)
            nc.vector.tensor_copy(ident_sb[:], ident_tmp[:])
            ones_sb = res.tile([1, P], f32)
            nc.vector.memset(ones_sb[:], 1.0)

            for g0, gs in blocks:
                nb = gs * P
                tr_ps = psum.tile([FH, nb], f32, tag="tr")
                for c in range(gs):
                    g = g0 + c
                    K = Ks[g]
                    off0 = sum(Ks[:g])
                    # HW indirect DMA supports exactly one offset per
                    # partition: K narrow gathers into disjoint 16-col
                    # slices (independent -> pipelined), then tree-reduce.
                    G = work.tile([P, K * FH], f32, tag=f"G{c}")
                    for k in range(K):
                        inst = nc.gpsimd.indirect_dma_start(
                            out=G[:, k * FH : (k + 1) * FH],
                            out_offset=None,
                            in_=table[:],
                            in_offset=bass.IndirectOffsetOnAxis(
                                ap=idxs_sb[:, off0 + k : off0 + k + 1], axis=0
                            ),
                        )
                        qi = n_gathers % 4
                        inst.ins.queue = f"qPoolDynamic{qi or ''}"
                        n_gathers += 1
                    w = K
                    while w > 1:
                        half = w // 2
                        nc.vector.tensor_add(
                            out=G[:, : half * FH],
                            in0=G[:, : half * FH],
                            in1=G[:, (w - half) * FH : w * FH],
                        )
                        w -= half
                    # scaled agg goes to a DVE-only tile so the PE transpose
                    # doesn't also need a wait on the gather's DMA semaphore
                    A = work.tile([P, FH], f32, tag="aggnm")
                    nc.vector.tensor_scalar_mul(
                        A[:],
                        G[:, :FH],
                        invdeg_sb[:, g : g + 1],
                    )
                    nc.tensor.transpose(
                        out=tr_ps[:, c * P : (c + 1) * P],
                        in_=A[:],
                        identity=ident_sb[:],
                    )
                agg_sb = mlp.tile([FH, nb], f32, tag="agg")
                nc.scalar.activation(agg_sb[:], tr_ps[:], AF.Copy)

                h_ps = psum.tile([HID, nb], f32, tag="h")
                col0 = g0 * P
                nc.tensor.matmul(
                    h_ps[:],
                    w1a_sb[:],
                    radial_sb[:, col0 : col0 + nb],
                    start=True,
                    stop=False,
                )
                nc.tensor.matmul(h_ps[:], w1b_sb[:], agg_sb[:], start=False, stop=True)
                h_sb = mlp.tile([HID, nb], f32, tag="h_sb")
                nc.scalar.activation(h_sb[:], h_ps[:], AF.Relu, bias=b1_sb[:, :1])

                o_ps = psum.tile([P, gs * F], f32, tag="o")
                for c in range(gs):
                    nc.tensor.matmul(
                        o_ps[:, c * F : (c + 1) * F],
                        ones_sb[:],
                        b2_sb[:],
                        start=True,
                        stop=False,
                    )
                    nc.tensor.matmul(
                        o_ps[:, c * F : (c + 1) * F],
                        h_sb[:, c * P : (c + 1) * P],
                        w2_sb[:],
                        start=False,
                        stop=True,
                    )
                o_sb = mlp.tile([P, gs * F], f32, tag="o_sb")
                nc.scalar.activation(o_sb[:], o_ps[:], AF.Copy)
                for c in range(gs):
                    nc.sync.dma_start(
                        out=out[col0 + c * P : col0 + (c + 1) * P, :],
                        in_=o_sb[:, c * F : (c + 1) * F],
                    )
    return nc


# ------------------------------------------------------------------ driver


def _run(x, edge_index, W1, b1, W2, b2, trace=False):
    from concourse.bass_utils import run_bass_kernel_spmd

    prep = _host_prep(x, edge_index)
    nc = build_program(prep["Ks"], prep["ncols"], prep["table"].shape[0])
    if not nc.is_finalized():
        nc.finalize()

    W1 = np.ascontiguousarray(W1, np.float32)
    in_maps = []
    for c in range(N_CORES):
        in_maps.append(
            {
                "table": prep["table"],
                "idxs": np.ascontiguousarray(prep["idx_all"][c]),
                "radial": np.ascontiguousarray(prep["radial_all"][c]),
                "invdeg": np.ascontiguousarray(prep["invdeg_all"][c]),
                "w1a": np.ascontiguousarray(W1[:FH]),
                "w1b": np.ascontiguousarray(W1[FH:]),
                "w2": np.ascontiguousarray(W2, np.float32),
                "b1": np.ascontiguousarray(b1, np.float32).reshape(HID, 1),
                "b2": np.ascontiguousarray(b2, np.float32).reshape(1, F),
            }
        )
    br = run_bass_kernel_spmd(
        nc, in_maps, list(range(N_CORES)), trace=trace
    )

    N = x.shape[0]
    n_groups = prep["n_groups"]
    ncols = prep["ncols"]
    order = prep["order"]
    result = np.empty((N, F), np.float32)
    r = np.arange(ncols)
    g = r // P
    p = r % P
    for c in range(N_CORES):
        shard = np.asarray(br.results[c]["out"])
        newid = GROUP * g + P * c + p
        valid = newid < N
        result[order[newid[valid]]] = shard[valid]
    return result, br


def kernel(x, edge_index, W1, b1, W2, b2):
    x = np.ascontiguousarray(np.asarray(x), np.float32)
    edge_index = np.ascontiguousarray(np.asarray(edge_index), np.int32)
    result, _ = _run(
        x,
        edge_index,
        np.asarray(W1),
        np.asarray(b1),
        np.asarray(W2),
        np.asarray(b2),
    )
    return result


# revision 12
# speedup vs baseline: 1.0082x; 1.0082x over previous
"""GNN mean-aggregation + 2-layer MLP on 8 Trainium2 NeuronCores.

Reference computation (see problem):
    rows = [i;j], cols = [j;i]                      (symmetrized COO)
    agg[n]  = mean over entries (n, c) of conical[c]   (deg clamped to 1)
    out     = relu([radial | agg] @ W1 + b1) @ W2 + b2

Strategy (nodes sharded 8 ways, MLP weights replicated):
  Host: symmetrize edges, degree-sort nodes (descending), build a padded
  CSR whose pad width K_g is uniform within each "group" of 8 tiles x 128
  nodes (one tile per core), so all 8 cores run the SAME program with
  identical shapes.  Per-core aux tensors: neighbor-index table
  idx[128, S], feature-major radial block, 1/deg, split W1.
  Device (per core): per 512-node block, one wide indirect-DMA gather
  [128, sum(K)*16] from the conical table (the memory-bound part), DVE
  tree-reduce to per-node sums, scale by 1/deg, PE-transpose to
  feature-major, MLP with stationary weights (bias via rank-1 matmul),
  relu on ACT, node-major DMA out.  Host inverse-permutes shards.
"""

import math

import numpy as np

N_CORES = 8
P = 128
GROUP = N_CORES * P  # 1024 nodes per group (one 128-tile per core)
FH = 16  # radial / conical half-width
F = 32
HID = 128
BLOCK_GROUPS = 4  # groups fused per gather instruction / MLP block


# ---------------------------------------------------------------- host prep


def _host_prep(x, edge_index):
    N = x.shape[0]
    i = edge_index[0].astype(np.int64)
    j = edge_index[1].astype(np.int64)
    rows = np.concatenate([i, j])
    cols = np.concatenate([j, i]).astype(np.int32)
    deg = np.bincount(rows, minlength=N)

    order = np.argsort(-deg, kind="stable")  # new id -> orig id
    deg_sorted = deg[order]

    eorder = np.argsort(rows, kind="stable")
    sorted_cols = cols[eorder]
    row_ptr = np.zeros(N + 1, np.int64)
    row_ptr[1:] = np.cumsum(deg)

    n_groups = math.ceil(N / GROUP)
    Ks = [max(int(deg_sorted[g * GROUP]), 1) for g in range(n_groups)]
    S = sum(Ks)
    ncols = n_groups * P  # rows per core shard

    idx_all = np.full((N_CORES, P, S), N, np.int32)  # N = zero row
    invdeg_all = np.ones((N_CORES, P, n_groups), np.float32)
    radial_all = np.zeros((N_CORES, FH, ncols), np.float32)

    total = sorted_cols.shape[0]
    off = 0
    for g in range(n_groups):
        K = Ks[g]
        lo = g * GROUP
        hi = min(lo + GROUP, N)
        n_real = hi - lo
        orig = order[lo:hi]
        d = deg_sorted[lo:hi]
        pos = row_ptr[orig][:, None] + np.arange(K)[None, :]
        mask = np.arange(K)[None, :] < d[:, None]
        vals = np.where(mask, sorted_cols[np.minimum(pos, total - 1)], N)
        blk = np.full((GROUP, K), N, np.int32)
        blk[:n_real] = vals
        idx_all[:, :, off : off + K] = blk.reshape(N_CORES, P, K)

        dd = np.ones((GROUP,), np.float32)
        dd[:n_real] = (1.0 / np.maximum(d, 1)).astype(np.float32)
        invdeg_all[:, :, g] = dd.reshape(N_CORES, P)

        rr = np.zeros((GROUP, FH), np.float32)
        rr[:n_real] = x[orig, :FH]
        radial_all[:, :, g * P : (g + 1) * P] = rr.reshape(N_CORES, P, FH).transpose(
            0, 2, 1
        )
        off += K

    table = np.concatenate([x[:, FH:F], np.zeros((1, FH), np.float32)], axis=0)
    table = np.ascontiguousarray(table, dtype=np.float32)
    return dict(
        order=order,
        Ks=Ks,
        S=S,
        n_groups=n_groups,
        ncols=ncols,
        idx_all=idx_all,
        invdeg_all=invdeg_all,
        radial_all=radial_all,
        table=table,
    )


# ------------------------------------------------------------- bass program


def build_program(Ks, ncols, table_rows):
    import concourse.bass as bass
    import concourse.tile as tile
    from concourse import bacc, mybir

    f32 = mybir.dt.float32
    i32 = mybir.dt.int32
    AF = mybir.ActivationFunctionType
    S = sum(Ks)
    n_groups = len(Ks)

    # Bacc (not raw Bass): its compile() splits multi-wait instructions into
    # event semaphores to satisfy TRN2's 1-wait-per-instruction constraint.
    nc = bacc.Bacc(None)
    table = nc.dram_tensor("table", [table_rows, FH], f32, kind="ExternalInput")
    idxs = nc.dram_tensor("idxs", [P, S], i32, kind="ExternalInput")
    radial = nc.dram_tensor("radial", [FH, ncols], f32, kind="ExternalInput")
    invdeg = nc.dram_tensor("invdeg", [P, n_groups], f32, kind="ExternalInput")
    w1a = nc.dram_tensor("w1a", [FH, HID], f32, kind="ExternalInput")
    w1b = nc.dram_tensor("w1b", [FH, HID], f32, kind="ExternalInput")
    w2 = nc.dram_tensor("w2", [HID, F], f32, kind="ExternalInput")
    b1 = nc.dram_tensor("b1", [HID, 1], f32, kind="ExternalInput")
    b2 = nc.dram_tensor("b2", [1, F], f32, kind="ExternalInput")
    out = nc.dram_tensor("out", [ncols, F], f32, kind="ExternalOutput")

    blocks = []
    g0 = 0
    while g0 < n_groups:
        gs = min(BLOCK_GROUPS, n_groups - g0)
        blocks.append((g0, gs))
        g0 += gs

    with tile.TileContext(nc) as tc:
        with (
            tc.tile_pool(name="res", bufs=1) as res,
            tc.tile_pool(name="work", bufs=3) as work,
            tc.tile_pool(name="mlp", bufs=2) as mlp,
            tc.tile_pool(name="psum", bufs=2, space="PSUM") as psum,
        ):
            idxs_sb = res.tile([P, S], i32)
            nc.sync.dma_start(out=idxs_sb[:], in_=idxs[:])
            radial_sb = res.tile([FH, ncols], f32)
            nc.sync.dma_start(out=radial_sb[:], in_=radial[:])
            invdeg_sb = res.tile([P, n_groups], f32)
            nc.sync.dma_start(out=invdeg_sb[:], in_=invdeg[:])
            w1a_sb = res.tile([FH, HID], f32)
            nc.sync.dma_start(out=w1a_sb[:], in_=w1a[:])
            w1b_sb = res.tile([FH, HID], f32)
            nc.sync.dma_start(out=w1b_sb[:], in_=w1b[:])
            w2_sb = res.tile([HID, F], f32)
            nc.sync.dma_start(out=w2_sb[:], in_=w2[:])
            b1_sb = res.tile([HID, 1], f32)
            nc.sync.dma_start(out=b1_sb[:], in_=b1[:])
            b2_sb = res.tile([1, F], f32)
            nc.sync.dma_start(out=b2_sb[:], in_=b2[:])
            # identity's final writer must be DVE so PE transposes carry one
            # DVE wait instead of an extra Pool wait (HW limit: 2 waits/inst)
            from concourse.masks import make_identity

            ident_tmp = res.tile([P, P], f32)
            make_identity(nc, ident_tmp[:])
            ident_sb = res.tile([P, P], f32)
            nc.vector.tensor_copy(ident_sb[:], ident_tmp[:])
            ones_sb = res.tile([1, P], f32)
            nc.vector.memset(ones_sb[:], 1.0)

            for g0, gs in blocks:
                nb = gs * P
                tr_ps = psum.tile([FH, nb], f32, tag="tr")
                for c in range(gs):
                    g = g0 + c
                    K = Ks[g]
                    off0 = sum(Ks[:g])
                    # HW indirect DMA supports exactly one offset per
                    # partition: K narrow gathers into disjoint 16-col
                    # slices (independent -> pipelined), then tree-reduce.
                    G = work.tile([P, K * FH], f32, tag=f"G{c}")
                    for k in range(K):
                        nc.gpsimd.indirect_dma_start(
                            out=G[:, k * FH : (k + 1) * FH],
                            out_offset=None,
                            in_=table[:],
                            in_offset=bass.IndirectOffsetOnAxis(
                                ap=idxs_sb[:, off0 + k : off0 + k + 1], axis=0
                            ),
                        )
                    w = K
                    while w > 1:
                        half = w // 2
                        nc.vector.tensor_add(
                            out=G[:, : half * FH],
                            in0=G[:, : half * FH],
                            in1=G[:, (w - half) * FH : w * FH],
                        )
                        w -= half
                    # scaled agg goes to a DVE-only tile so the PE transpose
                    # doesn't also need a wait on the gather's DMA semaphore
                    A = work.tile([P, FH], f32, tag="aggnm")
                    nc.vector.tensor_scalar_mul(
                        A[:],
                        G[:, :FH],
                        invdeg_sb[:, g : g + 1],
                    )
                    nc.tensor.transpose(
                        out=tr_ps[:, c * P : (c + 1) * P],
                        in_=A[:],
                        identity=ident_sb[:],
                    )
                agg_sb = mlp.tile([FH, nb], f32, tag="agg")
                nc.scalar.activation(agg_sb[:], tr_ps[:], AF.Copy)

                h_ps = psum.tile([HID, nb], f32, tag="h")
                col0 = g0 * P
                nc.tensor.matmul(
                    h_ps[:],
                    w1a_sb[:],
                    radial_sb[:, col0 : col0 + nb],
                    start=True,
                    stop=False,
                )
                nc.tensor.matmul(h_ps[:], w1b_sb[:], agg_sb[:], start=False, stop=True)
                h_sb = mlp.tile([HID, nb], f32, tag="h_sb")
                nc.scalar.activation(h_sb[:], h_ps[:], AF.Relu, bias=b1_sb[:, :1])

                o_ps = psum.tile([P, gs * F], f32, tag="o")
                for c in range(gs):
                    nc.tensor.matmul(
                        o_ps[:, c * F : (c + 1) * F],
                        ones_sb[:],
                        b2_sb[:],
                        start=True,
                        stop=False,
                    )
                    nc.tensor.matmul(
                        o_ps[:, c * F : (c + 1) * F],
                        h_sb[:, c * P : (c + 1) * P],
                        w2_sb[:],
                        start=False,
                        stop=True,
                    )
                o_sb = mlp.tile([P, gs * F], f32, tag="o_sb")
                nc.scalar.activation(o_sb[:], o_ps[:], AF.Copy)
                for c in range(gs):
                    nc.sync.dma_start(
                        out=out[col0 + c * P : col0 + (c + 1) * P, :],
                        in_=o_sb[:, c * F : (c + 1) * F],
                    )
    return nc


# ------------------------------------------------------------------ driver


def _run(x, edge_index, W1, b1, W2, b2, trace=False):
    from concourse.bass_utils import run_bass_kernel_spmd

    prep = _host_prep(x, edge_index)
    nc = build_program(prep["Ks"], prep["ncols"], prep["table"].shape[0])
    if not nc.is_finalized():
        nc.finalize()

    W1 = np.ascontiguousarray(W1, np.float32)
    in_maps = []
    for c in range(N_CORES):
        in_maps.append(
            {
                "table": prep["table"],
                "idxs": np.ascontiguousarray(prep["idx_all"][c]),
                "radial": np.ascontiguousarray(prep["radial_all"][c]),
                "invdeg": np.ascontiguousarray(prep["invdeg_all"][c]),
                "w1a": np.ascontiguousarray(W1[:FH]),
                "w1b": np.ascontiguousarray(W1[FH:]),
                "w2": np.ascontiguousarray(W2, np.float32),
                "b1": np.ascontiguousarray(b1, np.float32).reshape(HID, 1),
                "b2": np.ascontiguousarray(b2, np.float32).reshape(1, F),
            }
        )
    br = run_bass_kernel_spmd(
        nc, in_maps, list(range(N_CORES)), trace=trace
    )

    N = x.shape[0]
    n_groups = prep["n_groups"]
    ncols = prep["ncols"]
    order = prep["order"]
    result = np.empty((N, F), np.float32)
    r = np.arange(ncols)
    g = r // P
    p = r % P
    for c in range(N_CORES):
        shard = np.asarray(br.results[c]["out"])
        newid = GROUP * g + P * c + p
        valid = newid < N
        result[order[newid[valid]]] = shard[valid]
    return result, br


def kernel(x, edge_index, W1, b1, W2, b2):
    x = np.ascontiguousarray(np.asarray(x), np.float32)
    edge_index = np.ascontiguousarray(np.asarray(edge_index), np.int32)
    result, _ = _run(
        x,
        edge_index,
        np.asarray(W1),
        np.asarray(b1),
        np.asarray(W2),
        np.asarray(b2),
    )
    return result
